# revision 14
# baseline (speedup 1.0000x reference)
"""Trainium2 Bass kernel: 4-layer GPT-2-style transformer (B=2, S=2048, D=1024,
H=16, DH=64, M=4096, V=50257) on 8 NeuronCores.

v3 sharding (one SPMD program):
  - Token ownership: core c owns tokens [256c, 256c+256) of BOTH batches
    (local columns [0:256] = batch0, [256:512] = batch1).  Residual / LN /
    MLP / W_O / unembed are token-parallel on those 512 tokens.
  - Attention: head-parallel. Core c owns heads {2c, 2c+1}. Per layer the
    locally-computed q/k/v (all heads, own tokens) are routed with six
    AllToAlls (k/q/v x batch), attention runs per batch over all 2048
    keys, and z routes back with two more AllToAlls.
  - Attention matmuls are packed: score MMs for the 2 heads are row-packed
    (K=64 each at row groups 0/64), AV MMs col-packed (M=64 at col groups
    0/64).  Softmax denominators come from a DVE-accumulated exp-sum plus
    one tiny M=1 ones-matmul per (chunk, head).
  - Unembed: token-parallel over the full vocab (padded to 99*512=50688).
    lhsT (the x^T tile) is stationary across 4 vocab tiles, so LDWEIGHTS
    amortizes 4x.  W_U is pre-blocked host-side for contiguous DMA.
  - Embedding rows are gathered host-side (pure indexing = sharding);
    the device adds W_pos^T.
  - All matmul operands f16 (weights converted host-side); PSUM is f32.
"""

import sys, os
sys.path.insert(0, '/opt/trn_rl_repo')
os.environ.setdefault('MYCRO_LOCAL_CACHE', '1')

from contextlib import ExitStack

import numpy as np

import concourse.bass as bass
import concourse.bacc as bacc
import concourse.mybir as mybir
import concourse.tile as tile
from concourse.bass_utils import run_bass_kernel_spmd
from concourse.masks import make_identity

# model dims
B, S, V, D, H, DH, MLPD, L = 2, 2048, 50257, 1024, 16, 64, 4096, 4
EPS = 1e-5
NCORES = 8
TB = S // NCORES      # 256 tokens per batch per core
T = 2 * TB            # 512 local tokens
DT = D // 128         # 8 d-tiles
INV_SQRT_DH = float(1.0 / np.sqrt(DH))
NV = 99               # vocab tiles of 512 (50688 padded)
VP = NV * 512         # 50688
NKT = S // 128        # 16 key tiles per batch

F32 = mybir.dt.float32
F32R = mybir.dt.float32r
I32 = mybir.dt.int32
F16 = mybir.dt.float16
AF = mybir.ActivationFunctionType
OP = mybir.AluOpType

ALL8 = [[0, 1, 2, 3, 4, 5, 6, 7]]

_COMPILED = None


def ts(i, n):
    return slice(i * n, (i + 1) * n)


def _build():
    nc = bacc.Bacc("TRN2", target_bir_lowering=False, debug=False,
                   num_devices=NCORES)

    # ---------------- I/O -----------------
    # embedding rows (W_E[tok]) gathered host-side, transposed: [D, T]
    embT_d = nc.dram_tensor("embT", [D, T], F32, kind="ExternalInput")
    wposT_d = nc.dram_tensor("wposT", [D, TB], F32, kind="ExternalInput")
    # q/k/v weights, head-major cols: [L, 3(qkv), 8 ct, 8 k, 128, 128]
    wqkv_d = nc.dram_tensor("wqkv", [L, 3, DT, DT, 128, 128], F16,
                            kind="ExternalInput")
    wo_d = nc.dram_tensor("wo", [L, DT, DT, 128, 128], F16,
                          kind="ExternalInput")
    wi_d = nc.dram_tensor("wi", [L, MLPD // 128, DT, 128, 128], F16,
                          kind="ExternalInput")
    wout_d = nc.dram_tensor("wout", [L, DT, MLPD // 128, 128, 128], F16,
                            kind="ExternalInput")
    wu_d = nc.dram_tensor("wu", [NV, DT, 128, 512], F16,
                          kind="ExternalInput")
    logits_d = nc.dram_tensor("logits", [T, VP], F16, kind="ExternalOutput")

    # ------------- collective buffers (reused every layer) -------------
    kb = [nc.dram_tensor(f"kb{b}", [NCORES * 128, TB], F16) for b in range(2)]
    qb = [nc.dram_tensor(f"qb{b}", [NCORES * 128, TB], F16) for b in range(2)]
    vb = [nc.dram_tensor(f"vb{b}", [NCORES * 128, TB], F16) for b in range(2)]
    zb = [nc.dram_tensor(f"zb{b}", [NCORES * 128, TB], F16) for b in range(2)]
    kg = [nc.dram_tensor(f"kg{b}", [NCORES * 128, TB], F16)
          for b in range(2)]
    qg = [nc.dram_tensor(f"qg{b}", [NCORES * 128, TB], F16)
          for b in range(2)]
    vg = [nc.dram_tensor(f"vg{b}", [NCORES * 128, TB], F16)
          for b in range(2)]
    zg = [nc.dram_tensor(f"zg{b}", [NCORES * 128, TB], F16)
          for b in range(2)]

    def a2a(in_t, out_t):
        nc.gpsimd.collective_compute(
            "AllToAll", OP.bypass, replica_groups=ALL8,
            ins=[in_t[:]], outs=[out_t[:]])

    with tile.TileContext(nc) as tc:
        with tc.tile_pool(name="ps_mm", bufs=2, space="PSUM") as pps_mm, \
             tc.tile_pool(name="ps_sc", bufs=4, space="PSUM") as pps_sc, \
             tc.tile_pool(name="ps_z", bufs=2, space="PSUM") as pps_z:

          with ExitStack() as octx:
            # pools that live for the whole kernel
            p1 = octx.enter_context(tc.tile_pool(name="const", bufs=1))
            pxf = octx.enter_context(tc.tile_pool(name="pxf", bufs=DT))

            with ExitStack() as lctx:
                presid = lctx.enter_context(tc.tile_pool(name="presid",
                                                         bufs=DT))
                pxln = lctx.enter_context(tc.tile_pool(name="pxln", bufs=8))
                pqkv = lctx.enter_context(tc.tile_pool(name="pqkv", bufs=6))
                pzgt = lctx.enter_context(tc.tile_pool(name="pzgt", bufs=2))
                pbig = lctx.enter_context(tc.tile_pool(name="pbig", bufs=2))
                pva = lctx.enter_context(tc.tile_pool(name="pva", bufs=18))
                pex = lctx.enter_context(tc.tile_pool(name="pex", bufs=8))
                pxs = lctx.enter_context(tc.tile_pool(name="pxs", bufs=4))
                psq = lctx.enter_context(tc.tile_pool(name="psq", bufs=2))
                ppost = lctx.enter_context(tc.tile_pool(name="ppost",
                                                        bufs=17))
                pw = lctx.enter_context(tc.tile_pool(name="pw", bufs=3))
                pln = lctx.enter_context(tc.tile_pool(name="pln", bufs=8))
                prc = lctx.enter_context(tc.tile_pool(name="prc", bufs=2))
                ptmp = lctx.enter_context(tc.tile_pool(name="ptmp", bufs=2))

                # ---------- constants ----------
                ident = p1.tile([128, 128], F32, tag="ident")
                make_identity(nc, ident[:])
                ident16 = p1.tile([128, 128], F16, tag="ident16")
                nc.vector.tensor_copy(ident16[:], ident[:])
                onesf = p1.tile([128, 128], F32, tag="onesf")
                nc.vector.memset(onesf[:], 1.0)
                ones_c = p1.tile([128, 1], F32R, tag="ones_c")
                nc.vector.tensor_copy(ones_c[:], onesf[:, 0:1])
                ones_r64 = p1.tile([1, 64], F32R, tag="ones_r64")
                nc.vector.tensor_copy(ones_r64[:], onesf[0:1, 0:64])
                ones_r128 = p1.tile([1, 128], F32R, tag="ones_r128")
                nc.vector.tensor_copy(ones_r128[:], onesf[0:1, :])
                eps_t = p1.tile([1, 1], F32, tag="eps")
                nc.vector.memset(eps_t[:], EPS)
                # multiplicative causal masks for the 4 key tiles of a
                # diagonal 512-query chunk; mask[k, q] = 1 iff q >= k + off
                masks = []
                for mi in range(4):
                    off = 128 * mi
                    mk = p1.tile([128, 512], F16, tag=f"mask{mi}")
                    nc.gpsimd.memset(mk[:], 1.0)
                    nc.gpsimd.affine_select(
                        out=mk[:], in_=mk[:], compare_op=OP.is_ge,
                        fill=0.0, base=-off, pattern=[[1, 512]],
                        channel_multiplier=-1)
                    masks.append(mk)

                # residual stream x^T, [D on partitions, T tokens], f32r
                resid = [presid.tile([128, T], F32R, tag="resid",
                                     name=f"resid{i}") for i in range(DT)]

                def layer_norm(src_tiles, dst_tiles, cols=slice(0, T)):
                    """dst = (src - mean_D)/sqrt(var_D + eps) per token;
                    x^T layout, stats over partitions via ones-matmuls."""
                    n = cols.stop - cols.start
                    sum_ps = pps_mm.tile([1, n], F32, tag="mm")
                    sq_ps = pps_mm.tile([1, n], F32, tag="mm")
                    for d in range(DT):
                        sq = psq.tile([128, n], F32R, tag="sq")
                        nc.vector.tensor_tensor(
                            out=sq[:], in0=src_tiles[d][:, cols],
                            in1=src_tiles[d][:, cols], op=OP.mult)
                        nc.tensor.matmul(sum_ps[:], ones_c[:],
                                         src_tiles[d][:, cols],
                                         start=(d == 0), stop=(d == DT - 1))
                        nc.tensor.matmul(sq_ps[:], ones_c[:], sq[:],
                                         start=(d == 0), stop=(d == DT - 1))
                    mean = pln.tile([1, n], F32R, tag="ln")
                    nc.scalar.mul(mean[:], sum_ps[:], 1.0 / D)
                    ems = pln.tile([1, n], F32, tag="ln")
                    nc.scalar.mul(ems[:], sq_ps[:], 1.0 / D)
                    m2 = pln.tile([1, n], F32, tag="ln")
                    nc.scalar.activation(m2[:], mean[:], AF.Square)
                    var = pln.tile([1, n], F32, tag="ln")
                    nc.vector.tensor_tensor(out=var[:], in0=ems[:],
                                            in1=m2[:], op=OP.subtract)
                    lnv = pln.tile([1, n], F32, tag="ln")
                    nc.scalar.activation(lnv[:], var[:], AF.Ln, bias=eps_t[:])
                    rstd = pln.tile([1, n], F32R, tag="ln")
                    nc.scalar.activation(rstd[:], lnv[:], AF.Exp, scale=-0.5)
                    bc_m = pps_mm.tile([128, n], F32, tag="mm")
                    nc.tensor.matmul(bc_m[:], ones_r128[:], mean[:],
                                     start=True, stop=True)
                    bc_r = pps_mm.tile([128, n], F32, tag="mm")
                    nc.tensor.matmul(bc_r[:], ones_r128[:], rstd[:],
                                     start=True, stop=True)
                    for d in range(DT):
                        tmp = ptmp.tile([128, n], F32, tag="lntmp")
                        nc.vector.tensor_tensor(out=tmp[:],
                                                in0=src_tiles[d][:, cols],
                                                in1=bc_m[:], op=OP.subtract)
                        nc.vector.tensor_tensor(out=dst_tiles[d][:, cols],
                                                in0=tmp[:], in1=bc_r[:],
                                                op=OP.mult)

                # ================= embedding =================
                with nc.named_scope("embed"), \
                     tc.tile_pool(name="pemb", bufs=4) as pemb:
                    for d in range(DT):
                        wp = pemb.tile([128, TB], F32, tag="wpos")
                        nc.sync.dma_start(wp[:], wposT_d[ts(d, 128), :])
                        et = pemb.tile([128, T], F32, tag="emb")
                        nc.sync.dma_start(et[:], embT_d[ts(d, 128), :])
                        for h in range(2):
                            nc.vector.tensor_tensor(
                                out=resid[d][:, ts(h, TB)],
                                in0=et[:, ts(h, TB)], in1=wp[:], op=OP.add)

                # ================= layers =================
                for l in range(L):
                    # ---- LN1 + local qkv (all heads, my tokens) ----
                    with nc.named_scope(f"l{l}_qkv"):
                        xln = [pxln.tile([128, T], F16, tag="xln",
                                         name=f"xln_{l}_{i}")
                               for i in range(DT)]
                        layer_norm(resid, xln)
                        # proj order k, q, v; issue A2As as results land
                        for pi, (bounce, gath) in ((1, (kb, kg)),
                                                   (0, (qb, qg)),
                                                   (2, (vb, vg))):
                            for ct in range(DT):
                                w = pw.tile([128, DT, 128], F16, tag="w")
                                nc.sync.dma_start(
                                    w[:],
                                    wqkv_d[l, pi, ct].rearrange(
                                        "k p c -> p k c"))
                                ps = pps_mm.tile([128, T], F32, tag="mm")
                                for k in range(DT):
                                    nc.tensor.matmul(
                                        ps[:], w[:, k, :], xln[k][:],
                                        start=(k == 0), stop=(k == DT - 1))
                                ot = pqkv.tile([128, T], F16, tag="qkv")
                                nc.vector.tensor_copy(ot[:], ps[:])
                                for b2 in range(2):
                                    nc.sync.dma_start(
                                        bounce[b2][ts(ct, 128), :],
                                        ot[:, ts(b2, TB)])
                            a2a(bounce[0], gath[0])
                            a2a(bounce[1], gath[1])

                    # ---- attention per batch ----
                    zT = [pbig.tile([128, 8, TB], F16, tag="zT",
                                    name=f"zT{l}_{b2}") for b2 in range(2)]
                    for b2 in range(2):
                        with nc.named_scope(f"l{l}_attn{b2}"):
                            khp = pbig.tile([128, 8, TB], F16, tag="khp")
                            nc.sync.dma_start(
                                khp[:],
                                kg[b2][:].rearrange("(j p) t -> p j t",
                                                    p=128))
                            qhp = pbig.tile([128, 8, TB], F16, tag="qhp")
                            nc.sync.dma_start(
                                qhp[:],
                                qg[b2][:].rearrange("(j p) t -> p j t",
                                                    p=128))
                            vt = pbig.tile([128, 8, TB], F16, tag="vt")
                            nc.sync.dma_start(
                                vt[:],
                                vg[b2][:].rearrange("(j p) t -> p j t",
                                                    p=128))
                            # v -> normal layout [keys, 2*64] per key tile
                            va = []
                            for kt in range(NKT):
                                tp = pps_z.tile([128, 128], F16, tag="z")
                                nc.tensor.transpose(
                                    tp[:],
                                    vt[:, kt // 2, ts(kt % 2, 128)],
                                    ident16[:])
                                vat = pva.tile([128, 128], F16, tag="va",
                                               name=f"va{l}_{b2}_{kt}")
                                nc.vector.tensor_copy(vat[:], tp[:])
                                va.append(vat)
                            for g in range(4):
                                nk = 4 * g + 4
                                zps = pps_z.tile([128, 512], F32, tag="z")
                                exsum = [pxs.tile([128, 512], F32R,
                                                  tag="exs",
                                                  name=f"exs{hh}")
                                         for hh in range(2)]
                                for kt in range(nk):
                                    exs = []
                                    for hh in range(2):
                                        scps = pps_sc.tile([128, 512], F32,
                                                           tag="sc")
                                        nc.tensor.matmul(
                                            scps[:],
                                            khp[ts(hh, 64), kt // 2,
                                                ts(kt % 2, 128)],
                                            qhp[ts(hh, 64),
                                                2 * g:2 * g + 2, :],
                                            start=True, stop=True,
                                            tile_position=(64 * hh, 0))
                                        ex = pex.tile([128, 512], F16,
                                                      tag="ex")
                                        nc.scalar.activation(
                                            ex[:], scps[:], AF.Exp,
                                            scale=INV_SQRT_DH)
                                        if kt >= 4 * g:
                                            nc.vector.tensor_tensor(
                                                out=ex[:], in0=ex[:],
                                                in1=masks[kt - 4 * g][:],
                                                op=OP.mult)
                                        exs.append(ex)
                                    for hh in range(2):
                                        if kt == 0:
                                            nc.vector.tensor_copy(
                                                exsum[hh][:], exs[hh][:])
                                        else:
                                            nc.vector.tensor_tensor(
                                                out=exsum[hh][:],
                                                in0=exsum[hh][:],
                                                in1=exs[hh][:], op=OP.add)
                                        nc.tensor.matmul(
                                            zps[ts(hh, 64), :],
                                            va[kt][:, ts(hh, 64)],
                                            exs[hh][:],
                                            start=(kt == 0),
                                            stop=(kt == nk - 1),
                                            tile_position=(0, 64 * hh))
                                # denominators + scale
                                for hh in range(2):
                                    dn = pps_sc.tile([1, 512], F32,
                                                     tag="sc")
                                    nc.tensor.matmul(dn[:], ones_c[:],
                                                     exsum[hh][:],
                                                     start=True, stop=True)
                                    dnb = prc.tile([1, 512], F32, tag="dnb")
                                    nc.any.tensor_copy(dnb[:], dn[:])
                                    rc = prc.tile([1, 512], F32, tag="rc")
                                    nc.vector.reciprocal_approx_fast(
                                        out=rc[:], in_=dnb[:])
                                    rcr = prc.tile([1, 512], F32R,
                                                   tag="rcr")
                                    nc.vector.tensor_copy(rcr[:], rc[:])
                                    bc = pps_sc.tile([64, 512], F32,
                                                     tag="sc")
                                    nc.tensor.matmul(
                                        bc[:], ones_r64[:], rcr[:],
                                        start=True, stop=True)
                                    bcs = prc.tile([64, 512], F32,
                                                   tag="bcs")
                                    nc.vector.tensor_copy(bcs[:], bc[:])
                                    nc.vector.tensor_tensor(
                                        out=zT[b2][ts(hh, 64),
                                                   2 * g:2 * g + 2, :],
                                        in0=zps[ts(hh, 64), :].rearrange(
                                            "p (j t) -> p j t", t=TB),
                                        in1=bcs[:].rearrange(
                                            "p (j t) -> p j t", t=TB),
                                        op=OP.mult)
                                # route this chunk's z (2 owner cores)
                                for half in range(2):
                                    c8 = 2 * g + half
                                    nc.sync.dma_start(
                                        zb[b2][ts(c8, 128), :],
                                        zT[b2][:, 2 * g + half, :])
                            a2a(zb[b2], zg[b2])

                    # ---- W_O (token-parallel, per batch half) ----
                    for b2 in range(2):
                        with nc.named_scope(f"l{l}_wo{b2}"):
                            hc = ts(b2, TB)
                            zgt = pzgt.tile([128, DT, TB], F16, tag="zgt")
                            nc.sync.dma_start(
                                zgt[:],
                                zg[b2][:].rearrange("(j p) t -> p j t",
                                                    p=128))
                            for m in range(DT):
                                w = pw.tile([128, DT, 128], F16, tag="w")
                                nc.sync.dma_start(
                                    w[:],
                                    wo_d[l, m].rearrange("k p c -> p k c"))
                                ps = pps_mm.tile([128, TB], F32, tag="mm")
                                for k in range(DT):
                                    nc.tensor.matmul(
                                        ps[:], w[:, k, :], zgt[:, k, :],
                                        start=(k == 0), stop=(k == DT - 1))
                                nc.vector.tensor_tensor(
                                    out=resid[m][:, hc],
                                    in0=resid[m][:, hc], in1=ps[:],
                                    op=OP.add)

                    # ---- LN2 + MLP (full width) ----
                    with nc.named_scope(f"l{l}_mlp"):
                        xln2 = [pxln.tile([128, T], F16, tag="xln",
                                          name=f"xln2_{l}_{i}")
                                for i in range(DT)]
                        layer_norm(resid, xln2)
                        for half in range(2):
                            post = []
                            for mh in range(16):
                                m = 16 * half + mh
                                w = pw.tile([128, DT, 128], F16, tag="w")
                                nc.sync.dma_start(
                                    w[:],
                                    wi_d[l, m].rearrange("k p c -> p k c"))
                                pool_ = pps_mm if mh % 2 == 0 else pps_sc
                                ps = pool_.tile(
                                    [128, T], F32,
                                    tag="mm" if mh % 2 == 0 else "sc")
                                for k in range(DT):
                                    nc.tensor.matmul(
                                        ps[:], w[:, k, :], xln2[k][:],
                                        start=(k == 0), stop=(k == DT - 1))
                                po = ppost.tile([128, T], F16, tag="post")
                                nc.scalar.activation(po[:], ps[:],
                                                     AF.Gelu_apprx_tanh)
                                post.append(po)
                            for m in range(DT):
                                w = pw.tile([128, 16, 128], F16, tag="w2")
                                nc.sync.dma_start(
                                    w[:],
                                    wout_d[l, m, ts(half, 16)].rearrange(
                                        "k p c -> p k c"))
                                pool_ = pps_mm if m % 2 == 0 else pps_sc
                                ps = pool_.tile(
                                    [128, T], F32,
                                    tag="mm" if m % 2 == 0 else "sc")
                                for k in range(16):
                                    nc.tensor.matmul(
                                        ps[:], w[:, k, :], post[k][:],
                                        start=(k == 0), stop=(k == 15))
                                nc.vector.tensor_tensor(out=resid[m][:],
                                                        in0=resid[m][:],
                                                        in1=ps[:],
                                                        op=OP.add)

                # ---- final LN ----
                with nc.named_scope("final_ln"):
                    xf = [pxf.tile([128, T], F16, tag="xf", name=f"xf{i}")
                          for i in range(DT)]
                    layer_norm(resid, xf)

            # ================= unembed (token-parallel, full vocab) ======
            with nc.named_scope("unembed"), \
                 tc.tile_pool(name="uw", bufs=12) as puw, \
                 tc.tile_pool(name="uo", bufs=4) as puo:
                NG = 25                      # groups of 4 vocab tiles
                for ng in range(NG):
                    nj = 4 if ng < 24 else 3
                    wsb = []
                    for k in range(DT):
                        w = puw.tile([128, 4, 512], F16, tag="wu")
                        nc.sync.dma_start(
                            w[:, :nj, :],
                            wu_d[ts(ng, 4).start:ts(ng, 4).start + nj,
                                 k].rearrange("j p c -> p j c"))
                        wsb.append(w)
                    for tt in range(4):
                        if (ng * 4 + tt) % 2 == 0:
                            pj = [pps_sc.tile([128, 512], F32, tag="sc",
                                              name=f"pj{j}")
                                  for j in range(nj)]
                        else:
                            pj = [(pps_mm if j % 2 == 0 else pps_z).tile(
                                      [128, 512], F32,
                                      tag="mm" if j % 2 == 0 else "z",
                                      name=f"pj{j}")
                                  for j in range(nj)]
                        for k in range(DT):
                            for j in range(nj):
                                nc.tensor.matmul(
                                    pj[j][:], xf[k][:, ts(tt, 128)],
                                    wsb[k][:, j, :],
                                    start=(k == 0), stop=(k == DT - 1))
                        o = puo.tile([128, 4, 512], F16, tag="lgo")
                        for j in range(nj):
                            nc.any.tensor_copy(o[:, j, :], pj[j][:])
                        nc.sync.dma_start(
                            logits_d[ts(tt, 128),
                                     2048 * ng: 2048 * ng + 512 * nj]
                            .rearrange("p (j c) -> p j c", c=512),
                            o[:, :nj, :])

    nc.compile()
    return nc


def _prep_inputs(inputs):
    """Validate + build the 8 per-core input maps (host-side sharding)."""
    inp = {k: np.asarray(v) for k, v in inputs.items()}

    for name in ('b_Q', 'b_K', 'b_V', 'b_O', 'b_in', 'b_out', 'b_U',
                 'ln1_b', 'ln2_b', 'lnf_b'):
        if inp[name].any():
            raise NotImplementedError(f"nonzero {name} not supported")
    for name in ('ln1_w', 'ln2_w', 'lnf_w'):
        if not np.all(inp[name] == 1.0):
            raise NotImplementedError(f"non-unit {name} not supported")

    tokens = inp['tokens']                                        # [B, S]
    W_E = np.asarray(inp['W_E'], np.float32)                      # [V, D]
    W_posT = np.ascontiguousarray(inp['W_pos'].T, np.float32)     # [D, S]
    # head-major qkv: [L, D, H*DH] -> blocks [L, 3, ct, k, 128, 128]
    wqkv = np.empty((L, 3, DT, DT, 128, 128), np.float16)
    for pi, nm in ((0, 'W_Q'), (1, 'W_K'), (2, 'W_V')):
        wf = inp[nm].transpose(0, 2, 1, 3).reshape(L, D, H * DH)
        # block [l, ct, k, p, c] = wf[l, 128k+p, 128ct+c]
        wqkv[:, pi] = wf.reshape(L, DT, 128, DT, 128).transpose(0, 3, 1, 2, 4)
    WOf = inp['W_O'].reshape(L, H * DH, D)
    wo = np.ascontiguousarray(
        WOf.reshape(L, DT, 128, DT, 128).transpose(0, 3, 1, 2, 4),
        np.float16)
    WIf = inp['W_in']                                             # [L, D, M]
    wi = np.ascontiguousarray(
        WIf.reshape(L, DT, 128, MLPD // 128, 128).transpose(0, 3, 1, 2, 4),
        np.float16)
    WOUTf = inp['W_out']                                          # [L, M, D]
    wout = np.ascontiguousarray(
        WOUTf.reshape(L, MLPD // 128, 128, DT, 128).transpose(0, 3, 1, 2, 4),
        np.float16)
    WU = np.zeros((D, VP), np.float32)
    WU[:, :V] = inp['W_U']
    # wu block [n, k, p, c] = WU[128k+p, 512n+c]
    wu = np.ascontiguousarray(
        WU.reshape(DT, 128, NV, 512).transpose(2, 0, 1, 3), np.float16)

    in_maps = []
    for c in range(NCORES):
        sl = slice(TB * c, TB * (c + 1))
        toks = np.concatenate([tokens[0, sl], tokens[1, sl]])     # [512]
        embT = np.ascontiguousarray(W_E[toks].T)                  # [D, 512]
        in_maps.append({
            'embT': embT,
            'wposT': np.ascontiguousarray(W_posT[:, sl]),
            'wqkv': wqkv,
            'wo': wo,
            'wi': wi,
            'wout': wout,
            'wu': wu,
        })
    return in_maps


def kernel(**inputs):
    global _COMPILED
    if _COMPILED is None:
        _COMPILED = _build()
    nc = _COMPILED

    in_maps = _prep_inputs(inputs)
    trace = bool(int(os.environ.get('KERNEL_TRACE', '0')))
    res = run_bass_kernel_spmd(nc, in_maps, core_ids=list(range(NCORES)),
                               trace=trace)
    kernel.last_results = res

    logits = np.empty((B, S, V), np.float32)
    for c in range(NCORES):
        lg = res.results[c]['logits']                 # [512, VP] f16
        sl = slice(TB * c, TB * (c + 1))
        logits[0, sl] = lg[:TB, :V].astype(np.float32)
        logits[1, sl] = lg[TB:, :V].astype(np.float32)
    return logits


# revision 16
# speedup vs baseline: 1.0063x; 1.0063x over previous
"""Trainium2 Bass kernel: 4-layer GPT-2-style transformer (B=2, S=2048, D=1024,
H=16, DH=64, M=4096, V=50257) on 8 NeuronCores.

v3 sharding (one SPMD program):
  - Token ownership: core c owns tokens [256c, 256c+256) of BOTH batches
    (local columns [0:256] = batch0, [256:512] = batch1).  Residual / LN /
    MLP / W_O / unembed are token-parallel on those 512 tokens.
  - Attention: head-parallel. Core c owns heads {2c, 2c+1}. Per layer the
    locally-computed q/k/v (all heads, own tokens) are routed with six
    AllToAlls (k/q/v x batch), attention runs per batch over all 2048
    keys, and z routes back with two more AllToAlls.
  - Attention matmuls are packed: score MMs for the 2 heads are row-packed
    (K=64 each at row groups 0/64), AV MMs col-packed (M=64 at col groups
    0/64).  Softmax denominators come from a DVE-accumulated exp-sum plus
    one tiny M=1 ones-matmul per (chunk, head).
  - Unembed: token-parallel over the full vocab (padded to 99*512=50688).
    lhsT (the x^T tile) is stationary across 4 vocab tiles, so LDWEIGHTS
    amortizes 4x.  W_U is pre-blocked host-side for contiguous DMA.
  - Embedding rows are gathered host-side (pure indexing = sharding);
    the device adds W_pos^T.
  - All matmul operands f16 (weights converted host-side); PSUM is f32.
"""

import sys, os
sys.path.insert(0, '/opt/trn_rl_repo')
os.environ.setdefault('MYCRO_LOCAL_CACHE', '1')

from contextlib import ExitStack

import numpy as np

import concourse.bass as bass
import concourse.bacc as bacc
import concourse.mybir as mybir
import concourse.tile as tile
from concourse.bass_utils import run_bass_kernel_spmd
from concourse.masks import make_identity
# model dims
B, S, V, D, H, DH, MLPD, L = 2, 2048, 50257, 1024, 16, 64, 4096, 4
EPS = 1e-5
NCORES = 8
TB = S // NCORES      # 256 tokens per batch per core
T = 2 * TB            # 512 local tokens
DT = D // 128         # 8 d-tiles
INV_SQRT_DH = float(1.0 / np.sqrt(DH))
NV = 99               # vocab tiles of 512 (50688 padded)
VP = NV * 512         # 50688
NKT = S // 128        # 16 key tiles per batch

F32 = mybir.dt.float32
F32R = mybir.dt.float32r
I32 = mybir.dt.int32
F16 = mybir.dt.float16
AF = mybir.ActivationFunctionType
OP = mybir.AluOpType

ALL8 = [[0, 1, 2, 3, 4, 5, 6, 7]]

_COMPILED = None


def ts(i, n):
    return slice(i * n, (i + 1) * n)


def _build():
    nc = bacc.Bacc("TRN2", target_bir_lowering=False, debug=False,
                   num_devices=NCORES)

    # ---------------- I/O -----------------
    # embedding rows (W_E[tok]) gathered host-side, transposed: [D, T]
    embT_d = nc.dram_tensor("embT", [D, T], F32, kind="ExternalInput")
    wposT_d = nc.dram_tensor("wposT", [D, TB], F32, kind="ExternalInput")
    # q/k/v weights, head-major cols: [L, 3(qkv), 8 ct, 8 k, 128, 128]
    wqkv_d = nc.dram_tensor("wqkv", [L, 3, DT, DT, 128, 128], F16,
                            kind="ExternalInput")
    wo_d = nc.dram_tensor("wo", [L, DT, DT, 128, 128], F16,
                          kind="ExternalInput")
    wi_d = nc.dram_tensor("wi", [L, MLPD // 128, DT, 128, 128], F16,
                          kind="ExternalInput")
    wout_d = nc.dram_tensor("wout", [L, DT, MLPD // 128, 128, 128], F16,
                            kind="ExternalInput")
    wu_d = nc.dram_tensor("wu", [NV, DT, 128, 512], F16,
                          kind="ExternalInput")
    logits_d = nc.dram_tensor("logits", [T, VP], F16, kind="ExternalOutput")

    # ------------- collective buffers (reused every layer) -------------
    kb = [nc.dram_tensor(f"kb{b}", [NCORES * 128, TB], F16) for b in range(2)]
    qb = [nc.dram_tensor(f"qb{b}", [NCORES * 128, TB], F16) for b in range(2)]
    vb = [nc.dram_tensor(f"vb{b}", [NCORES * 128, TB], F16) for b in range(2)]
    zb = [nc.dram_tensor(f"zb{b}", [NCORES * 128, TB], F16) for b in range(2)]
    kg = [nc.dram_tensor(f"kg{b}", [NCORES * 128, TB], F16)
          for b in range(2)]
    qg = [nc.dram_tensor(f"qg{b}", [NCORES * 128, TB], F16)
          for b in range(2)]
    vg = [nc.dram_tensor(f"vg{b}", [NCORES * 128, TB], F16)
          for b in range(2)]
    zg = [nc.dram_tensor(f"zg{b}", [NCORES * 128, TB], F16)
          for b in range(2)]

    def a2a(in_t, out_t):
        nc.gpsimd.collective_compute(
            "AllToAll", OP.bypass, replica_groups=ALL8,
            ins=[in_t[:]], outs=[out_t[:]])

    with tile.TileContext(nc) as tc:
        with tc.tile_pool(name="ps_mm", bufs=2, space="PSUM") as pps_mm, \
             tc.tile_pool(name="ps_sc", bufs=4, space="PSUM") as pps_sc, \
             tc.tile_pool(name="ps_z", bufs=2, space="PSUM") as pps_z:

          with ExitStack() as octx:
            # pools that live for the whole kernel
            p1 = octx.enter_context(tc.tile_pool(name="const", bufs=1))
            pxf = octx.enter_context(tc.tile_pool(name="pxf", bufs=DT))

            with ExitStack() as lctx:
                presid = lctx.enter_context(tc.tile_pool(name="presid",
                                                         bufs=DT))
                pxln = lctx.enter_context(tc.tile_pool(name="pxln", bufs=8))
                pqkv = lctx.enter_context(tc.tile_pool(name="pqkv", bufs=6))
                pzgt = lctx.enter_context(tc.tile_pool(name="pzgt", bufs=2))
                pbig = lctx.enter_context(tc.tile_pool(name="pbig", bufs=2))
                pva = lctx.enter_context(tc.tile_pool(name="pva", bufs=18))
                pex = lctx.enter_context(tc.tile_pool(name="pex", bufs=8))
                pxs = lctx.enter_context(tc.tile_pool(name="pxs", bufs=4))
                psq = lctx.enter_context(tc.tile_pool(name="psq", bufs=2))
                ppost = lctx.enter_context(tc.tile_pool(name="ppost",
                                                        bufs=17))
                pw = lctx.enter_context(tc.tile_pool(name="pw", bufs=3))
                pln = lctx.enter_context(tc.tile_pool(name="pln", bufs=8))
                prc = lctx.enter_context(tc.tile_pool(name="prc", bufs=2))
                ptmp = lctx.enter_context(tc.tile_pool(name="ptmp", bufs=2))

                # ---------- constants ----------
                ident = p1.tile([128, 128], F32, tag="ident")
                make_identity(nc, ident[:])
                ident16 = p1.tile([128, 128], F16, tag="ident16")
                nc.vector.tensor_copy(ident16[:], ident[:])
                onesf = p1.tile([128, 128], F32, tag="onesf")
                nc.vector.memset(onesf[:], 1.0)
                ones_c = p1.tile([128, 1], F32R, tag="ones_c")
                nc.vector.tensor_copy(ones_c[:], onesf[:, 0:1])
                ones_r64 = p1.tile([1, 64], F32R, tag="ones_r64")
                nc.vector.tensor_copy(ones_r64[:], onesf[0:1, 0:64])
                ones_r128 = p1.tile([1, 128], F32R, tag="ones_r128")
                nc.vector.tensor_copy(ones_r128[:], onesf[0:1, :])
                eps_t = p1.tile([1, 1], F32, tag="eps")
                nc.vector.memset(eps_t[:], EPS)
                # multiplicative causal masks for the 4 key tiles of a
                # diagonal 512-query chunk; mask[k, q] = 1 iff q >= k + off
                masks = []
                for mi in range(4):
                    off = 128 * mi
                    mk = p1.tile([128, 512], F16, tag=f"mask{mi}")
                    nc.gpsimd.memset(mk[:], 1.0)
                    nc.gpsimd.affine_select(
                        out=mk[:], in_=mk[:], compare_op=OP.is_ge,
                        fill=0.0, base=-off, pattern=[[1, 512]],
                        channel_multiplier=-1)
                    masks.append(mk)

                # residual stream x^T, [D on partitions, T tokens], f32r
                resid = [presid.tile([128, T], F32R, tag="resid",
                                     name=f"resid{i}") for i in range(DT)]

                def layer_norm(src_tiles, dst_tiles, cols=slice(0, T)):
                    """dst = (src - mean_D)/sqrt(var_D + eps) per token;
                    x^T layout, stats over partitions via ones-matmuls."""
                    n = cols.stop - cols.start
                    sum_ps = pps_mm.tile([1, n], F32, tag="mm")
                    sq_ps = pps_mm.tile([1, n], F32, tag="mm")
                    for d in range(DT):
                        sq = psq.tile([128, n], F32R, tag="sq")
                        nc.vector.tensor_tensor(
                            out=sq[:], in0=src_tiles[d][:, cols],
                            in1=src_tiles[d][:, cols], op=OP.mult)
                        nc.tensor.matmul(sum_ps[:], ones_c[:],
                                         src_tiles[d][:, cols],
                                         start=(d == 0), stop=(d == DT - 1))
                        nc.tensor.matmul(sq_ps[:], ones_c[:], sq[:],
                                         start=(d == 0), stop=(d == DT - 1))
                    mean = pln.tile([1, n], F32R, tag="ln")
                    nc.scalar.mul(mean[:], sum_ps[:], 1.0 / D)
                    ems = pln.tile([1, n], F32, tag="ln")
                    nc.scalar.mul(ems[:], sq_ps[:], 1.0 / D)
                    m2 = pln.tile([1, n], F32, tag="ln")
                    nc.scalar.activation(m2[:], mean[:], AF.Square)
                    var = pln.tile([1, n], F32, tag="ln")
                    nc.vector.tensor_tensor(out=var[:], in0=ems[:],
                                            in1=m2[:], op=OP.subtract)
                    lnv = pln.tile([1, n], F32, tag="ln")
                    nc.scalar.activation(lnv[:], var[:], AF.Ln, bias=eps_t[:])
                    rstd = pln.tile([1, n], F32R, tag="ln")
                    nc.scalar.activation(rstd[:], lnv[:], AF.Exp, scale=-0.5)
                    bc_m = pps_mm.tile([128, n], F32, tag="mm")
                    nc.tensor.matmul(bc_m[:], ones_r128[:], mean[:],
                                     start=True, stop=True)
                    bc_r = pps_mm.tile([128, n], F32, tag="mm")
                    nc.tensor.matmul(bc_r[:], ones_r128[:], rstd[:],
                                     start=True, stop=True)
                    for d in range(DT):
                        tmp = ptmp.tile([128, n], F32, tag="lntmp")
                        nc.vector.tensor_tensor(out=tmp[:],
                                                in0=src_tiles[d][:, cols],
                                                in1=bc_m[:], op=OP.subtract)
                        nc.vector.tensor_tensor(out=dst_tiles[d][:, cols],
                                                in0=tmp[:], in1=bc_r[:],
                                                op=OP.mult)

                # ================= embedding =================
                with nc.named_scope("embed"), \
                     tc.tile_pool(name="pemb", bufs=4) as pemb:
                    for d in range(DT):
                        wp = pemb.tile([128, TB], F32, tag="wpos")
                        nc.sync.dma_start(wp[:], wposT_d[ts(d, 128), :])
                        et = pemb.tile([128, T], F32, tag="emb")
                        nc.sync.dma_start(et[:], embT_d[ts(d, 128), :])
                        for h in range(2):
                            nc.vector.tensor_tensor(
                                out=resid[d][:, ts(h, TB)],
                                in0=et[:, ts(h, TB)], in1=wp[:], op=OP.add)

                # ================= layers =================
                for l in range(L):
                    # ---- LN1 + local qkv (all heads, my tokens) ----
                    with nc.named_scope(f"l{l}_qkv"):
                        xln = [pxln.tile([128, T], F16, tag="xln",
                                         name=f"xln_{l}_{i}")
                               for i in range(DT)]
                        layer_norm(resid, xln)
                        # proj order k, q, v; issue A2As as results land
                        for pi, (bounce, gath) in ((1, (kb, kg)),
                                                   (0, (qb, qg)),
                                                   (2, (vb, vg))):
                            for ct in range(DT):
                                w = pw.tile([128, DT, 128], F16, tag="w")
                                nc.sync.dma_start(
                                    w[:],
                                    wqkv_d[l, pi, ct].rearrange(
                                        "k p c -> p k c"))
                                ps = pps_mm.tile([128, T], F32, tag="mm")
                                for k in range(DT):
                                    nc.tensor.matmul(
                                        ps[:], w[:, k, :], xln[k][:],
                                        start=(k == 0), stop=(k == DT - 1))
                                ot = pqkv.tile([128, T], F16, tag="qkv")
                                nc.vector.tensor_copy(ot[:], ps[:])
                                for b2 in range(2):
                                    nc.sync.dma_start(
                                        bounce[b2][ts(ct, 128), :],
                                        ot[:, ts(b2, TB)])
                            a2a(bounce[0], gath[0])
                            a2a(bounce[1], gath[1])

                    # ---- attention per batch ----
                    zT = [pbig.tile([128, 8, TB], F16, tag="zT",
                                    name=f"zT{l}_{b2}") for b2 in range(2)]
                    for b2 in range(2):
                        with nc.named_scope(f"l{l}_attn{b2}"):
                            khp = pbig.tile([128, 8, TB], F16, tag="khp")
                            nc.sync.dma_start(
                                khp[:],
                                kg[b2][:].rearrange("(j p) t -> p j t",
                                                    p=128))
                            qhp = pbig.tile([128, 8, TB], F16, tag="qhp")
                            nc.sync.dma_start(
                                qhp[:],
                                qg[b2][:].rearrange("(j p) t -> p j t",
                                                    p=128))
                            vt = pbig.tile([128, 8, TB], F16, tag="vt")
                            nc.sync.dma_start(
                                vt[:],
                                vg[b2][:].rearrange("(j p) t -> p j t",
                                                    p=128))
                            # v -> normal layout [keys, 2*64] per key tile
                            va = []
                            for kt in range(NKT):
                                tp = pps_z.tile([128, 128], F16, tag="z")
                                nc.tensor.transpose(
                                    tp[:],
                                    vt[:, kt // 2, ts(kt % 2, 128)],
                                    ident16[:])
                                vat = pva.tile([128, 128], F16, tag="va",
                                               name=f"va{l}_{b2}_{kt}")
                                nc.vector.tensor_copy(vat[:], tp[:])
                                va.append(vat)
                            def finalize(fin):
                                zps, exsum, g = fin
                                for hh in range(2):
                                    dn = pps_sc.tile([1, 512], F32,
                                                     tag="sc")
                                    nc.tensor.matmul(dn[:], ones_c[:],
                                                     exsum[hh][:],
                                                     start=True, stop=True)
                                    dnb = prc.tile([1, 512], F32, tag="dnb")
                                    nc.any.tensor_copy(dnb[:], dn[:])
                                    rc = prc.tile([1, 512], F32, tag="rc")
                                    nc.vector.reciprocal_approx_fast(
                                        out=rc[:], in_=dnb[:])
                                    rcr = prc.tile([1, 512], F32R,
                                                   tag="rcr")
                                    nc.vector.tensor_copy(rcr[:], rc[:])
                                    bc = pps_sc.tile([64, 512], F32,
                                                     tag="sc")
                                    nc.tensor.matmul(
                                        bc[:], ones_r64[:], rcr[:],
                                        start=True, stop=True)
                                    bcs = prc.tile([64, 512], F32,
                                                   tag="bcs")
                                    nc.vector.tensor_copy(bcs[:], bc[:])
                                    nc.vector.tensor_tensor(
                                        out=zT[b2][ts(hh, 64),
                                                   2 * g:2 * g + 2, :],
                                        in0=zps[ts(hh, 64), :].rearrange(
                                            "p (j t) -> p j t", t=TB),
                                        in1=bcs[:].rearrange(
                                            "p (j t) -> p j t", t=TB),
                                        op=OP.mult)
                                # route this chunk's z (2 owner cores)
                                for half in range(2):
                                    c8 = 2 * g + half
                                    nc.sync.dma_start(
                                        zb[b2][ts(c8, 128), :],
                                        zT[b2][:, 2 * g + half, :])

                            pend = None
                            for g in range(4):
                                nk = 4 * g + 4
                                zps = pps_z.tile([128, 512], F32, tag="z")
                                exsum = [pxs.tile([128, 512], F32R,
                                                  tag="exs",
                                                  name=f"exs{hh}")
                                         for hh in range(2)]
                                prev = None
                                for kt in range(nk + 1):
                                    exs = []
                                    if kt < nk:
                                        for hh in range(2):
                                            scps = pps_sc.tile(
                                                [128, 512], F32, tag="sc")
                                            nc.tensor.matmul(
                                                scps[:],
                                                khp[ts(hh, 64), kt // 2,
                                                    ts(kt % 2, 128)],
                                                qhp[ts(hh, 64),
                                                    2 * g:2 * g + 2, :],
                                                start=True, stop=True,
                                                tile_position=(64 * hh, 0))
                                            ex = pex.tile([128, 512], F16,
                                                          tag="ex")
                                            nc.scalar.activation(
                                                ex[:], scps[:], AF.Exp,
                                                scale=INV_SQRT_DH)
                                            if kt >= 4 * g:
                                                nc.vector.tensor_tensor(
                                                    out=ex[:], in0=ex[:],
                                                    in1=masks[kt - 4 * g][:],
                                                    op=OP.mult)
                                            exs.append(ex)
                                    if kt >= 1:
                                        pex_, pkt = prev, kt - 1
                                        for hh in range(2):
                                            nc.tensor.matmul(
                                                zps[ts(hh, 64), :],
                                                va[pkt][:, ts(hh, 64)],
                                                pex_[hh][:],
                                                start=(pkt == 0),
                                                stop=(pkt == nk - 1),
                                                tile_position=(0, 64 * hh))
                                    if kt < nk:
                                        for hh in range(2):
                                            if kt == 0:
                                                nc.vector.tensor_copy(
                                                    exsum[hh][:],
                                                    exs[hh][:])
                                            else:
                                                nc.vector.tensor_tensor(
                                                    out=exsum[hh][:],
                                                    in0=exsum[hh][:],
                                                    in1=exs[hh][:],
                                                    op=OP.add)
                                    if kt == 1 and pend is not None:
                                        finalize(pend)
                                        pend = None
                                    prev = exs
                                pend = (zps, exsum, g)
                            finalize(pend)
                            a2a(zb[b2], zg[b2])

                    # ---- W_O (token-parallel, per batch half) ----
                    for b2 in range(2):
                        with nc.named_scope(f"l{l}_wo{b2}"):
                            hc = ts(b2, TB)
                            zgt = pzgt.tile([128, DT, TB], F16, tag="zgt")
                            nc.sync.dma_start(
                                zgt[:],
                                zg[b2][:].rearrange("(j p) t -> p j t",
                                                    p=128))
                            for m in range(DT):
                                w = pw.tile([128, DT, 128], F16, tag="w")
                                nc.sync.dma_start(
                                    w[:],
                                    wo_d[l, m].rearrange("k p c -> p k c"))
                                ps = pps_mm.tile([128, TB], F32, tag="mm")
                                for k in range(DT):
                                    nc.tensor.matmul(
                                        ps[:], w[:, k, :], zgt[:, k, :],
                                        start=(k == 0), stop=(k == DT - 1))
                                nc.vector.tensor_tensor(
                                    out=resid[m][:, hc],
                                    in0=resid[m][:, hc], in1=ps[:],
                                    op=OP.add)

                    # ---- LN2 + MLP (full width) ----
                    with nc.named_scope(f"l{l}_mlp"):
                        xln2 = [pxln.tile([128, T], F16, tag="xln",
                                          name=f"xln2_{l}_{i}")
                                for i in range(DT)]
                        layer_norm(resid, xln2)
                        for half in range(2):
                            post = []
                            for mh in range(16):
                                m = 16 * half + mh
                                w = pw.tile([128, DT, 128], F16, tag="w")
                                nc.sync.dma_start(
                                    w[:],
                                    wi_d[l, m].rearrange("k p c -> p k c"))
                                pool_ = pps_mm if mh % 2 == 0 else pps_sc
                                ps = pool_.tile(
                                    [128, T], F32,
                                    tag="mm" if mh % 2 == 0 else "sc")
                                for k in range(DT):
                                    nc.tensor.matmul(
                                        ps[:], w[:, k, :], xln2[k][:],
                                        start=(k == 0), stop=(k == DT - 1))
                                po = ppost.tile([128, T], F16, tag="post")
                                nc.scalar.activation(po[:], ps[:],
                                                     AF.Gelu_apprx_tanh)
                                post.append(po)
                            for m in range(DT):
                                w = pw.tile([128, 16, 128], F16, tag="w2")
                                nc.sync.dma_start(
                                    w[:],
                                    wout_d[l, m, ts(half, 16)].rearrange(
                                        "k p c -> p k c"))
                                pool_ = pps_mm if m % 2 == 0 else pps_sc
                                ps = pool_.tile(
                                    [128, T], F32,
                                    tag="mm" if m % 2 == 0 else "sc")
                                for k in range(16):
                                    nc.tensor.matmul(
                                        ps[:], w[:, k, :], post[k][:],
                                        start=(k == 0), stop=(k == 15))
                                nc.vector.tensor_tensor(out=resid[m][:],
                                                        in0=resid[m][:],
                                                        in1=ps[:],
                                                        op=OP.add)

                # ---- final LN ----
                with nc.named_scope("final_ln"):
                    xf = [pxf.tile([128, T], F16, tag="xf", name=f"xf{i}")
                          for i in range(DT)]
                    layer_norm(resid, xf)

            # ================= unembed (token-parallel, full vocab) ======
            with nc.named_scope("unembed"), \
                 tc.tile_pool(name="uw", bufs=12) as puw, \
                 tc.tile_pool(name="uo", bufs=4) as puo:
                NG = 25                      # groups of 4 vocab tiles
                for ng in range(NG):
                    nj = 4 if ng < 24 else 3
                    wsb = []
                    for k in range(DT):
                        w = puw.tile([128, 4, 512], F16, tag="wu")
                        nc.sync.dma_start(
                            w[:, :nj, :],
                            wu_d[ts(ng, 4).start:ts(ng, 4).start + nj,
                                 k].rearrange("j p c -> p j c"))
                        wsb.append(w)
                    for tt in range(4):
                        if (ng * 4 + tt) % 2 == 0:
                            pj = [pps_sc.tile([128, 512], F32, tag="sc",
                                              name=f"pj{j}")
                                  for j in range(nj)]
                        else:
                            pj = [(pps_mm if j % 2 == 0 else pps_z).tile(
                                      [128, 512], F32,
                                      tag="mm" if j % 2 == 0 else "z",
                                      name=f"pj{j}")
                                  for j in range(nj)]
                        for k in range(DT):
                            for j in range(nj):
                                nc.tensor.matmul(
                                    pj[j][:], xf[k][:, ts(tt, 128)],
                                    wsb[k][:, j, :],
                                    start=(k == 0), stop=(k == DT - 1))
                        o = puo.tile([128, 4, 512], F16, tag="lgo")
                        for j in range(nj):
                            nc.any.tensor_copy(o[:, j, :], pj[j][:])
                        nc.sync.dma_start(
                            logits_d[ts(tt, 128),
                                     2048 * ng: 2048 * ng + 512 * nj]
                            .rearrange("p (j c) -> p j c", c=512),
                            o[:, :nj, :])

    nc.compile()
    return nc


def _prep_inputs(inputs):
    """Validate + build the 8 per-core input maps (host-side sharding)."""
    inp = {k: np.asarray(v) for k, v in inputs.items()}

    for name in ('b_Q', 'b_K', 'b_V', 'b_O', 'b_in', 'b_out', 'b_U',
                 'ln1_b', 'ln2_b', 'lnf_b'):
        if inp[name].any():
            raise NotImplementedError(f"nonzero {name} not supported")
    for name in ('ln1_w', 'ln2_w', 'lnf_w'):
        if not np.all(inp[name] == 1.0):
            raise NotImplementedError(f"non-unit {name} not supported")

    tokens = inp['tokens']                                        # [B, S]
    W_E = np.asarray(inp['W_E'], np.float32)                      # [V, D]
    W_posT = np.ascontiguousarray(inp['W_pos'].T, np.float32)     # [D, S]
    # head-major qkv: [L, D, H*DH] -> blocks [L, 3, ct, k, 128, 128]
    wqkv = np.empty((L, 3, DT, DT, 128, 128), np.float16)
    for pi, nm in ((0, 'W_Q'), (1, 'W_K'), (2, 'W_V')):
        wf = inp[nm].transpose(0, 2, 1, 3).reshape(L, D, H * DH)
        # block [l, ct, k, p, c] = wf[l, 128k+p, 128ct+c]
        wqkv[:, pi] = wf.reshape(L, DT, 128, DT, 128).transpose(0, 3, 1, 2, 4)
    WOf = inp['W_O'].reshape(L, H * DH, D)
    wo = np.ascontiguousarray(
        WOf.reshape(L, DT, 128, DT, 128).transpose(0, 3, 1, 2, 4),
        np.float16)
    WIf = inp['W_in']                                             # [L, D, M]
    wi = np.ascontiguousarray(
        WIf.reshape(L, DT, 128, MLPD // 128, 128).transpose(0, 3, 1, 2, 4),
        np.float16)
    WOUTf = inp['W_out']                                          # [L, M, D]
    wout = np.ascontiguousarray(
        WOUTf.reshape(L, MLPD // 128, 128, DT, 128).transpose(0, 3, 1, 2, 4),
        np.float16)
    WU = np.zeros((D, VP), np.float32)
    WU[:, :V] = inp['W_U']
    # wu block [n, k, p, c] = WU[128k+p, 512n+c]
    wu = np.ascontiguousarray(
        WU.reshape(DT, 128, NV, 512).transpose(2, 0, 1, 3), np.float16)

    in_maps = []
    for c in range(NCORES):
        sl = slice(TB * c, TB * (c + 1))
        toks = np.concatenate([tokens[0, sl], tokens[1, sl]])     # [512]
        embT = np.ascontiguousarray(W_E[toks].T)                  # [D, 512]
        in_maps.append({
            'embT': embT,
            'wposT': np.ascontiguousarray(W_posT[:, sl]),
            'wqkv': wqkv,
            'wo': wo,
            'wi': wi,
            'wout': wout,
            'wu': wu,
        })
    return in_maps


def kernel(**inputs):
    global _COMPILED
    if _COMPILED is None:
        _COMPILED = _build()
    nc = _COMPILED

    in_maps = _prep_inputs(inputs)
    trace = bool(int(os.environ.get('KERNEL_TRACE', '0')))
    res = run_bass_kernel_spmd(nc, in_maps, core_ids=list(range(NCORES)),
                               trace=trace)
    kernel.last_results = res

    logits = np.empty((B, S, V), np.float32)
    for c in range(NCORES):
        lg = res.results[c]['logits']                 # [512, VP] f16
        sl = slice(TB * c, TB * (c + 1))
        logits[0, sl] = lg[:TB, :V].astype(np.float32)
        logits[1, sl] = lg[TB:, :V].astype(np.float32)
    return logits


# revision 19
# speedup vs baseline: 1.0614x; 1.0547x over previous
"""Trainium2 Bass kernel: 4-layer GPT-2-style transformer (B=2, S=2048, D=1024,
H=16, DH=64, M=4096, V=50257) on 8 NeuronCores.

v3 sharding (one SPMD program):
  - Token ownership: core c owns tokens [256c, 256c+256) of BOTH batches
    (local columns [0:256] = batch0, [256:512] = batch1).  Residual / LN /
    MLP / W_O / unembed are token-parallel on those 512 tokens.
  - Attention: head-parallel. Core c owns heads {2c, 2c+1}. Per layer the
    locally-computed q/k/v (all heads, own tokens) are routed with six
    AllToAlls (k/q/v x batch), attention runs per batch over all 2048
    keys, and z routes back with two more AllToAlls.
  - Attention matmuls are packed: score MMs for the 2 heads are row-packed
    (K=64 each at row groups 0/64), AV MMs col-packed (M=64 at col groups
    0/64).  Softmax denominators come from a DVE-accumulated exp-sum plus
    one tiny M=1 ones-matmul per (chunk, head).
  - Unembed: token-parallel over the full vocab (padded to 99*512=50688).
    lhsT (the x^T tile) is stationary across 4 vocab tiles, so LDWEIGHTS
    amortizes 4x.  W_U is pre-blocked host-side for contiguous DMA.
  - Embedding rows are gathered host-side (pure indexing = sharding);
    the device adds W_pos^T.
  - All matmul operands f16 (weights converted host-side); PSUM is f32.
"""

import sys, os
sys.path.insert(0, '/opt/trn_rl_repo')
os.environ.setdefault('MYCRO_LOCAL_CACHE', '1')

from contextlib import ExitStack

import numpy as np

import concourse.bass as bass
import concourse.bacc as bacc
import concourse.mybir as mybir
import concourse.tile as tile
from concourse.bass_utils import run_bass_kernel_spmd
from concourse.masks import make_identity
# model dims
B, S, V, D, H, DH, MLPD, L = 2, 2048, 50257, 1024, 16, 64, 4096, 4
EPS = 1e-5
NCORES = 8
TB = S // NCORES      # 256 tokens per batch per core
T = 2 * TB            # 512 local tokens
DT = D // 128         # 8 d-tiles
INV_SQRT_DH = float(1.0 / np.sqrt(DH))
NV = 99               # vocab tiles of 512 (50688 padded)
VP = NV * 512         # 50688
NKT = S // 128        # 16 key tiles per batch

F32 = mybir.dt.float32
F32R = mybir.dt.float32r
I32 = mybir.dt.int32
F16 = mybir.dt.float16
AF = mybir.ActivationFunctionType
OP = mybir.AluOpType

ALL8 = [[0, 1, 2, 3, 4, 5, 6, 7]]

_COMPILED = None


def ts(i, n):
    return slice(i * n, (i + 1) * n)


def _build():
    nc = bacc.Bacc("TRN2", target_bir_lowering=False, debug=False,
                   num_devices=NCORES)

    # ---------------- I/O -----------------
    # embedding rows (W_E[tok]) gathered host-side, transposed: [D, T]
    embT_d = nc.dram_tensor("embT", [D, T], F32, kind="ExternalInput")
    wposT_d = nc.dram_tensor("wposT", [D, TB], F32, kind="ExternalInput")
    # q/k/v weights, head-major cols: [L, 3(qkv), 8 ct, 8 k, 128, 128]
    wqkv_d = nc.dram_tensor("wqkv", [L, 3, DT, DT, 128, 128], F16,
                            kind="ExternalInput")
    wo_d = nc.dram_tensor("wo", [L, DT, DT, 128, 128], F16,
                          kind="ExternalInput")
    wi_d = nc.dram_tensor("wi", [L, MLPD // 128, DT, 128, 128], F16,
                          kind="ExternalInput")
    wout_d = nc.dram_tensor("wout", [L, DT, MLPD // 128, 128, 128], F16,
                            kind="ExternalInput")
    wu_d = nc.dram_tensor("wu", [NV, DT, 128, 512], F16,
                          kind="ExternalInput")
    logits_d = nc.dram_tensor("logits", [T, VP], F16, kind="ExternalOutput")

    # ------------- collective buffers (reused every layer) -------------
    kqvb = [nc.dram_tensor(f"kqvb{b}", [NCORES * 384, TB], F16)
            for b in range(2)]
    kqvg = [nc.dram_tensor(f"kqvg{b}", [NCORES * 384, TB], F16)
            for b in range(2)]
    zb = [nc.dram_tensor(f"zb{b}", [NCORES * 128, TB], F16) for b in range(2)]
    zg = [nc.dram_tensor(f"zg{b}", [NCORES * 128, TB], F16)
          for b in range(2)]

    def a2a(in_t, out_t):
        nc.gpsimd.collective_compute(
            "AllToAll", OP.bypass, replica_groups=ALL8,
            ins=[in_t[:]], outs=[out_t[:]])

    with tile.TileContext(nc) as tc:
        with tc.tile_pool(name="ps_mm", bufs=2, space="PSUM") as pps_mm, \
             tc.tile_pool(name="ps_sc", bufs=2, space="PSUM") as pps_sc, \
             tc.tile_pool(name="ps_z", bufs=2, space="PSUM") as pps_z:

          with ExitStack() as octx:
            # pools that live for the whole kernel
            p1 = octx.enter_context(tc.tile_pool(name="const", bufs=1))
            pxf = octx.enter_context(tc.tile_pool(name="pxf", bufs=DT))

            with ExitStack() as lctx:
                presid = lctx.enter_context(tc.tile_pool(name="presid",
                                                         bufs=DT))
                pxln = lctx.enter_context(tc.tile_pool(name="pxln", bufs=8))
                pqkv = lctx.enter_context(tc.tile_pool(name="pqkv", bufs=6))
                pzgt = lctx.enter_context(tc.tile_pool(name="pzgt", bufs=2))
                pbig = lctx.enter_context(tc.tile_pool(name="pbig", bufs=2))
                pva = lctx.enter_context(tc.tile_pool(name="pva", bufs=18))
                pex = lctx.enter_context(tc.tile_pool(name="pex", bufs=6))
                pxs = lctx.enter_context(tc.tile_pool(name="pxs", bufs=2))
                psq = lctx.enter_context(tc.tile_pool(name="psq", bufs=2))
                ppost = lctx.enter_context(tc.tile_pool(name="ppost",
                                                        bufs=17))
                pw = lctx.enter_context(tc.tile_pool(name="pw", bufs=3))
                pln = lctx.enter_context(tc.tile_pool(name="pln", bufs=8))
                prc = lctx.enter_context(tc.tile_pool(name="prc", bufs=2))
                ptmp = lctx.enter_context(tc.tile_pool(name="ptmp", bufs=2))

                # ---------- constants ----------
                ident = p1.tile([128, 128], F32, tag="ident")
                make_identity(nc, ident[:])
                ident16 = p1.tile([128, 128], F16, tag="ident16")
                nc.vector.tensor_copy(ident16[:], ident[:])
                onesf = p1.tile([128, 128], F32, tag="onesf")
                nc.vector.memset(onesf[:], 1.0)
                ones_c = p1.tile([128, 1], F32R, tag="ones_c")
                nc.vector.tensor_copy(ones_c[:], onesf[:, 0:1])
                ones_r64 = p1.tile([1, 64], F32R, tag="ones_r64")
                nc.vector.tensor_copy(ones_r64[:], onesf[0:1, 0:64])
                ones_r128 = p1.tile([1, 128], F32R, tag="ones_r128")
                nc.vector.tensor_copy(ones_r128[:], onesf[0:1, :])
                eps_t = p1.tile([1, 1], F32, tag="eps")
                nc.vector.memset(eps_t[:], EPS)
                # multiplicative causal masks for the 4 key tiles of a
                # diagonal 512-query chunk; mask[k, q] = 1 iff q >= k + off
                masks = []
                for mi in range(4):
                    off = 128 * mi
                    mk = p1.tile([128, 1024], F16, tag=f"mask{mi}")
                    nc.gpsimd.memset(mk[:], 1.0)
                    for half in range(2):
                        nc.gpsimd.affine_select(
                            out=mk[:, ts(half, 512)],
                            in_=mk[:, ts(half, 512)], compare_op=OP.is_ge,
                            fill=0.0, base=-off, pattern=[[1, 512]],
                            channel_multiplier=-1)
                    masks.append(mk)

                # residual stream x^T, [D on partitions, T tokens], f32r
                resid = [presid.tile([128, T], F32R, tag="resid",
                                     name=f"resid{i}") for i in range(DT)]

                def layer_norm(src_tiles, dst_tiles, cols=slice(0, T)):
                    """dst = (src - mean_D)/sqrt(var_D + eps) per token;
                    x^T layout, stats over partitions via ones-matmuls."""
                    n = cols.stop - cols.start
                    sum_ps = pps_mm.tile([1, n], F32, tag="mm")
                    sq_ps = pps_mm.tile([1, n], F32, tag="mm")
                    for d in range(DT):
                        sq = psq.tile([128, n], F32R, tag="sq")
                        nc.vector.tensor_tensor(
                            out=sq[:], in0=src_tiles[d][:, cols],
                            in1=src_tiles[d][:, cols], op=OP.mult)
                        nc.tensor.matmul(sum_ps[:], ones_c[:],
                                         src_tiles[d][:, cols],
                                         start=(d == 0), stop=(d == DT - 1))
                        nc.tensor.matmul(sq_ps[:], ones_c[:], sq[:],
                                         start=(d == 0), stop=(d == DT - 1))
                    mean = pln.tile([1, n], F32R, tag="ln")
                    nc.scalar.mul(mean[:], sum_ps[:], 1.0 / D)
                    ems = pln.tile([1, n], F32, tag="ln")
                    nc.scalar.mul(ems[:], sq_ps[:], 1.0 / D)
                    m2 = pln.tile([1, n], F32, tag="ln")
                    nc.scalar.activation(m2[:], mean[:], AF.Square)
                    var = pln.tile([1, n], F32, tag="ln")
                    nc.vector.tensor_tensor(out=var[:], in0=ems[:],
                                            in1=m2[:], op=OP.subtract)
                    lnv = pln.tile([1, n], F32, tag="ln")
                    nc.scalar.activation(lnv[:], var[:], AF.Ln, bias=eps_t[:])
                    rstd = pln.tile([1, n], F32R, tag="ln")
                    nc.scalar.activation(rstd[:], lnv[:], AF.Exp, scale=-0.5)
                    bc_m = pps_mm.tile([128, n], F32, tag="mm")
                    nc.tensor.matmul(bc_m[:], ones_r128[:], mean[:],
                                     start=True, stop=True)
                    bc_r = pps_mm.tile([128, n], F32, tag="mm")
                    nc.tensor.matmul(bc_r[:], ones_r128[:], rstd[:],
                                     start=True, stop=True)
                    for d in range(DT):
                        tmp = ptmp.tile([128, n], F32, tag="lntmp")
                        nc.vector.tensor_tensor(out=tmp[:],
                                                in0=src_tiles[d][:, cols],
                                                in1=bc_m[:], op=OP.subtract)
                        nc.vector.tensor_tensor(out=dst_tiles[d][:, cols],
                                                in0=tmp[:], in1=bc_r[:],
                                                op=OP.mult)

                # ================= embedding =================
                with nc.named_scope("embed"), \
                     tc.tile_pool(name="pemb", bufs=3) as pemb:
                    for d in range(DT):
                        wp = pemb.tile([128, TB], F32, tag="wpos")
                        nc.sync.dma_start(wp[:], wposT_d[ts(d, 128), :])
                        et = pemb.tile([128, T], F32, tag="emb")
                        nc.sync.dma_start(et[:], embT_d[ts(d, 128), :])
                        for h in range(2):
                            nc.vector.tensor_tensor(
                                out=resid[d][:, ts(h, TB)],
                                in0=et[:, ts(h, TB)], in1=wp[:], op=OP.add)

                # ================= layers =================
                for l in range(L):
                    # ---- LN1 + local qkv (all heads, my tokens) ----
                    with nc.named_scope(f"l{l}_qkv"):
                        xln = [pxln.tile([128, T], F16, tag="xln",
                                         name=f"xln_{l}_{i}")
                               for i in range(DT)]
                        layer_norm(resid, xln)
                        # kqv bounce rows per shard ct: [k|q|v] x 128
                        for ct in range(DT):
                            for ri, pi in ((0, 1), (1, 0), (2, 2)):
                                w = pw.tile([128, DT, 128], F16, tag="w")
                                nc.sync.dma_start(
                                    w[:],
                                    wqkv_d[l, pi, ct].rearrange(
                                        "k p c -> p k c"))
                                ps = pps_mm.tile([128, T], F32, tag="mm")
                                for k in range(DT):
                                    nc.tensor.matmul(
                                        ps[:], w[:, k, :], xln[k][:],
                                        start=(k == 0), stop=(k == DT - 1))
                                ot = pqkv.tile([128, T], F16, tag="qkv")
                                nc.vector.tensor_copy(ot[:], ps[:])
                                for b2 in range(2):
                                    nc.sync.dma_start(
                                        kqvb[b2][384 * ct + 128 * ri:
                                                 384 * ct + 128 * (ri + 1),
                                                 :],
                                        ot[:, ts(b2, TB)])
                        a2a(kqvb[0], kqvg[0])
                        a2a(kqvb[1], kqvg[1])

                    # ---- attention per batch ----
                    zT = [pbig.tile([128, 8, TB], F16, tag="zT",
                                    name=f"zT{l}_{b2}") for b2 in range(2)]
                    for b2 in range(2):
                        with nc.named_scope(f"l{l}_attn{b2}"):
                            kqv = pbig.tile([128, 8, 3, TB], F16,
                                            tag="kqv")
                            nc.sync.dma_start(
                                kqv[:],
                                kqvg[b2][:].rearrange(
                                    "(j r p) t -> p j r t", r=3, p=128))
                            # v -> normal layout [keys, 2*64] per key tile
                            va = []
                            for kt in range(NKT):
                                tp = pps_z.tile([128, 128], F16, tag="z")
                                nc.tensor.transpose(
                                    tp[:],
                                    kqv[:, kt // 2, 2, ts(kt % 2, 128)],
                                    ident16[:])
                                vat = pva.tile([128, 128], F16, tag="va",
                                               name=f"va{l}_{b2}_{kt}")
                                nc.vector.tensor_copy(vat[:], tp[:])
                                va.append(vat)
                            def finalize(fin):
                                zps, exsum, g = fin
                                for hh in range(2):
                                    dn = pps_mm.tile([1, 512], F32,
                                                     tag="mm")
                                    nc.tensor.matmul(dn[:], ones_c[:],
                                                     exsum[:, ts(hh, 512)],
                                                     start=True, stop=True)
                                    dnb = prc.tile([1, 512], F32, tag="dnb")
                                    nc.any.tensor_copy(dnb[:], dn[:])
                                    rc = prc.tile([1, 512], F32, tag="rc")
                                    nc.vector.reciprocal_approx_fast(
                                        out=rc[:], in_=dnb[:])
                                    rcr = prc.tile([1, 512], F32R,
                                                   tag="rcr")
                                    nc.vector.tensor_copy(rcr[:], rc[:])
                                    bc = pps_mm.tile([64, 512], F32,
                                                     tag="mm")
                                    nc.tensor.matmul(
                                        bc[:], ones_r64[:], rcr[:],
                                        start=True, stop=True)
                                    bcs = prc.tile([64, 512], F32,
                                                   tag="bcs")
                                    nc.vector.tensor_copy(bcs[:], bc[:])
                                    nc.vector.tensor_tensor(
                                        out=zT[b2][ts(hh, 64),
                                                   2 * g:2 * g + 2, :],
                                        in0=zps[ts(hh, 64), :].rearrange(
                                            "p (j t) -> p j t", t=TB),
                                        in1=bcs[:].rearrange(
                                            "p (j t) -> p j t", t=TB),
                                        op=OP.mult)
                                # route this chunk's z (2 owner cores)
                                for half in range(2):
                                    c8 = 2 * g + half
                                    nc.sync.dma_start(
                                        zb[b2][ts(c8, 128), :],
                                        zT[b2][:, 2 * g + half, :])

                            pend = None
                            for g in range(4):
                                nk = 4 * g + 4
                                zps = pps_z.tile([128, 512], F32, tag="z")
                                exsum = pxs.tile([128, 1024], F32R,
                                                 tag="exs")
                                prev = None
                                for kt in range(nk + 1):
                                    exd = None
                                    if kt < nk:
                                        scps = pps_sc.tile(
                                            [128, 1024], F32, tag="sc")
                                        for hh in range(2):
                                            nc.tensor.matmul(
                                                scps[:, ts(hh, 512)],
                                                kqv[ts(hh, 64), kt // 2,
                                                    0, ts(kt % 2, 128)],
                                                kqv[ts(hh, 64),
                                                    2 * g:2 * g + 2, 1, :],
                                                start=True, stop=True,
                                                tile_position=(64 * hh, 0))
                                        exd = pex.tile([128, 1024], F16,
                                                       tag="ex")
                                        nc.scalar.activation(
                                            exd[:], scps[:], AF.Exp,
                                            scale=INV_SQRT_DH)
                                        if kt >= 4 * g:
                                            nc.vector.tensor_tensor(
                                                out=exd[:], in0=exd[:],
                                                in1=masks[kt - 4 * g][:],
                                                op=OP.mult)
                                    if kt >= 1:
                                        pex_, pkt = prev, kt - 1
                                        for hh in range(2):
                                            nc.tensor.matmul(
                                                zps[ts(hh, 64), :],
                                                va[pkt][:, ts(hh, 64)],
                                                pex_[:, ts(hh, 512)],
                                                start=(pkt == 0),
                                                stop=(pkt == nk - 1),
                                                tile_position=(0, 64 * hh))
                                    if kt < nk:
                                        if kt == 0:
                                            nc.vector.tensor_copy(
                                                exsum[:], exd[:])
                                        else:
                                            nc.vector.tensor_tensor(
                                                out=exsum[:],
                                                in0=exsum[:],
                                                in1=exd[:], op=OP.add)
                                    if kt == 1 and pend is not None:
                                        finalize(pend)
                                        pend = None
                                    prev = exd
                                pend = (zps, exsum, g)
                            finalize(pend)
                            a2a(zb[b2], zg[b2])

                    # ---- W_O (token-parallel, per batch half) ----
                    for b2 in range(2):
                        with nc.named_scope(f"l{l}_wo{b2}"):
                            hc = ts(b2, TB)
                            zgt = pzgt.tile([128, DT, TB], F16, tag="zgt")
                            nc.sync.dma_start(
                                zgt[:],
                                zg[b2][:].rearrange("(j p) t -> p j t",
                                                    p=128))
                            for m in range(DT):
                                w = pw.tile([128, DT, 128], F16, tag="w")
                                nc.sync.dma_start(
                                    w[:],
                                    wo_d[l, m].rearrange("k p c -> p k c"))
                                ps = pps_mm.tile([128, TB], F32, tag="mm")
                                for k in range(DT):
                                    nc.tensor.matmul(
                                        ps[:], w[:, k, :], zgt[:, k, :],
                                        start=(k == 0), stop=(k == DT - 1))
                                nc.vector.tensor_tensor(
                                    out=resid[m][:, hc],
                                    in0=resid[m][:, hc], in1=ps[:],
                                    op=OP.add)

                    # ---- LN2 + MLP (full width) ----
                    with nc.named_scope(f"l{l}_mlp"):
                        xln2 = [pxln.tile([128, T], F16, tag="xln",
                                          name=f"xln2_{l}_{i}")
                                for i in range(DT)]
                        layer_norm(resid, xln2)
                        for half in range(2):
                            post = []
                            for mh in range(16):
                                m = 16 * half + mh
                                w = pw.tile([128, DT, 128], F16, tag="w")
                                nc.sync.dma_start(
                                    w[:],
                                    wi_d[l, m].rearrange("k p c -> p k c"))
                                pool_ = pps_mm if mh % 2 == 0 else pps_sc
                                ps = pool_.tile(
                                    [128, T], F32,
                                    tag="mm" if mh % 2 == 0 else "sc")
                                for k in range(DT):
                                    nc.tensor.matmul(
                                        ps[:], w[:, k, :], xln2[k][:],
                                        start=(k == 0), stop=(k == DT - 1))
                                po = ppost.tile([128, T], F16, tag="post")
                                nc.scalar.activation(po[:], ps[:],
                                                     AF.Gelu_apprx_tanh)
                                post.append(po)
                            for m in range(DT):
                                w = pw.tile([128, 16, 128], F16, tag="w2")
                                nc.sync.dma_start(
                                    w[:],
                                    wout_d[l, m, ts(half, 16)].rearrange(
                                        "k p c -> p k c"))
                                pool_ = pps_mm if m % 2 == 0 else pps_sc
                                ps = pool_.tile(
                                    [128, T], F32,
                                    tag="mm" if m % 2 == 0 else "sc")
                                for k in range(16):
                                    nc.tensor.matmul(
                                        ps[:], w[:, k, :], post[k][:],
                                        start=(k == 0), stop=(k == 15))
                                nc.vector.tensor_tensor(out=resid[m][:],
                                                        in0=resid[m][:],
                                                        in1=ps[:],
                                                        op=OP.add)

                # ---- final LN ----
                with nc.named_scope("final_ln"):
                    xf = [pxf.tile([128, T], F16, tag="xf", name=f"xf{i}")
                          for i in range(DT)]
                    layer_norm(resid, xf)

            # ================= unembed (token-parallel, full vocab) ======
            with nc.named_scope("unembed"), \
                 tc.tile_pool(name="uw", bufs=12) as puw, \
                 tc.tile_pool(name="uo", bufs=4) as puo:
                NG = 25                      # groups of 4 vocab tiles
                for ng in range(NG):
                    nj = 4 if ng < 24 else 3
                    wsb = []
                    for k in range(DT):
                        w = puw.tile([128, 4, 512], F16, tag="wu")
                        nc.sync.dma_start(
                            w[:, :nj, :],
                            wu_d[ts(ng, 4).start:ts(ng, 4).start + nj,
                                 k].rearrange("j p c -> p j c"))
                        wsb.append(w)
                    for tt in range(4):
                        if (ng * 4 + tt) % 2 == 0:
                            pj = [pps_sc.tile([128, 512], F32, tag="sc",
                                              name=f"pj{j}")
                                  for j in range(nj)]
                        else:
                            pj = [(pps_mm if j % 2 == 0 else pps_z).tile(
                                      [128, 512], F32,
                                      tag="mm" if j % 2 == 0 else "z",
                                      name=f"pj{j}")
                                  for j in range(nj)]
                        for k in range(DT):
                            for j in range(nj):
                                nc.tensor.matmul(
                                    pj[j][:], xf[k][:, ts(tt, 128)],
                                    wsb[k][:, j, :],
                                    start=(k == 0), stop=(k == DT - 1))
                        o = puo.tile([128, 4, 512], F16, tag="lgo")
                        for j in range(nj):
                            nc.any.tensor_copy(o[:, j, :], pj[j][:])
                        nc.sync.dma_start(
                            logits_d[ts(tt, 128),
                                     2048 * ng: 2048 * ng + 512 * nj]
                            .rearrange("p (j c) -> p j c", c=512),
                            o[:, :nj, :])

    nc.compile()
    return nc


def _prep_inputs(inputs):
    """Validate + build the 8 per-core input maps (host-side sharding)."""
    inp = {k: np.asarray(v) for k, v in inputs.items()}

    for name in ('b_Q', 'b_K', 'b_V', 'b_O', 'b_in', 'b_out', 'b_U',
                 'ln1_b', 'ln2_b', 'lnf_b'):
        if inp[name].any():
            raise NotImplementedError(f"nonzero {name} not supported")
    for name in ('ln1_w', 'ln2_w', 'lnf_w'):
        if not np.all(inp[name] == 1.0):
            raise NotImplementedError(f"non-unit {name} not supported")

    tokens = inp['tokens']                                        # [B, S]
    W_E = np.asarray(inp['W_E'], np.float32)                      # [V, D]
    W_posT = np.ascontiguousarray(inp['W_pos'].T, np.float32)     # [D, S]
    # head-major qkv: [L, D, H*DH] -> blocks [L, 3, ct, k, 128, 128]
    wqkv = np.empty((L, 3, DT, DT, 128, 128), np.float16)
    for pi, nm in ((0, 'W_Q'), (1, 'W_K'), (2, 'W_V')):
        wf = inp[nm].transpose(0, 2, 1, 3).reshape(L, D, H * DH)
        # block [l, ct, k, p, c] = wf[l, 128k+p, 128ct+c]
        wqkv[:, pi] = wf.reshape(L, DT, 128, DT, 128).transpose(0, 3, 1, 2, 4)
    WOf = inp['W_O'].reshape(L, H * DH, D)
    wo = np.ascontiguousarray(
        WOf.reshape(L, DT, 128, DT, 128).transpose(0, 3, 1, 2, 4),
        np.float16)
    WIf = inp['W_in']                                             # [L, D, M]
    wi = np.ascontiguousarray(
        WIf.reshape(L, DT, 128, MLPD // 128, 128).transpose(0, 3, 1, 2, 4),
        np.float16)
    WOUTf = inp['W_out']                                          # [L, M, D]
    wout = np.ascontiguousarray(
        WOUTf.reshape(L, MLPD // 128, 128, DT, 128).transpose(0, 3, 1, 2, 4),
        np.float16)
    WU = np.zeros((D, VP), np.float32)
    WU[:, :V] = inp['W_U']
    # wu block [n, k, p, c] = WU[128k+p, 512n+c]
    wu = np.ascontiguousarray(
        WU.reshape(DT, 128, NV, 512).transpose(2, 0, 1, 3), np.float16)

    in_maps = []
    for c in range(NCORES):
        sl = slice(TB * c, TB * (c + 1))
        toks = np.concatenate([tokens[0, sl], tokens[1, sl]])     # [512]
        embT = np.ascontiguousarray(W_E[toks].T)                  # [D, 512]
        in_maps.append({
            'embT': embT,
            'wposT': np.ascontiguousarray(W_posT[:, sl]),
            'wqkv': wqkv,
            'wo': wo,
            'wi': wi,
            'wout': wout,
            'wu': wu,
        })
    return in_maps


def kernel(**inputs):
    global _COMPILED
    if _COMPILED is None:
        _COMPILED = _build()
    nc = _COMPILED

    in_maps = _prep_inputs(inputs)
    trace = bool(int(os.environ.get('KERNEL_TRACE', '0')))
    res = run_bass_kernel_spmd(nc, in_maps, core_ids=list(range(NCORES)),
                               trace=trace)
    kernel.last_results = res

    logits = np.empty((B, S, V), np.float32)
    for c in range(NCORES):
        lg = res.results[c]['logits']                 # [512, VP] f16
        sl = slice(TB * c, TB * (c + 1))
        logits[0, sl] = lg[:TB, :V].astype(np.float32)
        logits[1, sl] = lg[TB:, :V].astype(np.float32)
    return logits
